# revision 34
# baseline (speedup 1.0000x reference)
"""ConvLRU Trainium2 kernel (8-core SPMD).

Math: the reference chain x -> fft2 -> Wb -> gamma -> L-scan -> ifft2 -> Wc
-> real is linear; nothing between FFT_W and IFFT_W depends on the W-frequency
index, so the W-axis transforms cancel for the main output (the frequency-
domain bias bb becomes bb*delta_{w=0}, which pre-FFT_H is bb*delta_{h=0,w=0}).
Only last_hidden needs an FFT along W, done host-side on the tiny [B,1,C,H,W]
final state.

Sharding: core = b*4 + wg; each core owns 8 of 32 w-columns of one batch
element. Everything is core-local except LayerNorm statistics (AllReduce of
[1,64] per group of 4 cores).

Device layouts per (core, l):
  x_l    [c=128, (h,w8)=256]                    (direct DMA)
  h1     [(h_half,w8)=128, 256=(re o | im o)]   (Wb matmul, data as lhsT)
  z,y    same layout per h-half                 (FFT via block-delta G lhsT)
  z2     [o=128, 512=(re (h,w) | im (h,w))]     (IFFT, data as lhsT)
  out    [c=128, (h,w8)=256]
"""
import os
import sys

sys.path.insert(0, "/opt/trn_rl_repo")
import numpy as np

_DEBUG = bool(os.environ.get("K_DEBUG"))
_SIM = bool(os.environ.get("K_SIM"))

B, L, C, H, W = 2, 32, 128, 32, 32
W8 = 8
N_CORES = 8
LN_N = float(C * H * W)

_PATCHED = False


def _patch_tail_drain():
    """Split the Tile tail-drain's sem waits across single-wait SP nops —
    this walrus build rejects >2 sync waits on an SP instruction."""
    global _PATCHED
    if _PATCHED:
        return
    from concourse import tile
    from concourse.vector_clock import ScopedClock, VectorClock

    def _dab(self, tick_clock, wait_clock):
        gc = tick_clock.global_clock
        n = len(gc)
        for p in range(n):
            t = gc[p]
            if t > 0:
                vec = [0] * n
                vec[p] = t
                nop = self.nc.sync.nop(nofuse=True, hint=f"drain_split_{p}")
                wait_clock.add_sem_waits(
                    nop.ins, ScopedClock({None: VectorClock(vec)})
                )
        drain_inst = self.nc.sync.drain()
        wait_clock.add_sem_waits(
            drain_inst.ins,
            ScopedClock({None: gc.copy()}),
            ScopedClock({None: gc.copy()}),
        )
        self.nc.all_engine_barrier()
        assert self.sems is not None
        popped = self.nc._tile_sem_poison_stack.pop()
        assert popped is self._sem_poison
        self.nc.clear_and_free_semaphores(list(self.sems.allocated().values()))
        self.nc.all_engine_barrier()

    tile.TileContext._drain_and_barrier = _dab
    _PATCHED = True


def _cap_sync_waits(nc, limit=1):
    """Walrus in this container rejects instructions with more than ~2 sync
    waits. Move excess on_wait entries onto same-engine NoOps inserted just
    before the offending instruction (per-engine program order preserved)."""
    from concourse import mybir

    n_split = 0
    for fn in nc.m.functions:
        for bb in fn.blocks:
            insts = list(bb.instructions)
            out = []
            changed = False
            for ins in insts:
                si = ins.sync_info
                if si is not None and si.on_wait and len(si.on_wait) > limit:
                    waits = list(si.on_wait)
                    head, tail = waits[:-limit], waits[-limit:]
                    for i in range(0, len(head), limit):
                        n_split += 1
                        nop = mybir.InstNoOp(
                            name=f"wsplit_{n_split}",
                            sync_info=mybir.SyncInfo(
                                on_wait=head[i : i + limit], on_update=[]
                            ),
                            bass_nofuse=True,
                            engine=ins.engine,
                        )
                        out.append(nop)
                    ins.sync_info = mybir.SyncInfo(
                        on_wait=tail, on_update=list(si.on_update)
                    )
                    changed = True
                out.append(ins)
            if changed:
                bb.instructions = out
    return n_split


def _build_nc(ln_trivial=False, use_f32r=True):
    from concourse import bass, tile, mybir

    f32 = mybir.dt.float32
    f32w = mybir.dt.float32r if use_f32r else f32
    nc = bass.Bass(num_devices=N_CORES)

    xin = nc.dram_tensor("x", [L, 128, 256], f32w, kind="ExternalInput")
    wbc = nc.dram_tensor("wbc", [128, 256], f32w, kind="ExternalInput")
    wbcsw = nc.dram_tensor("wbcsw", [128, 256], f32w, kind="ExternalInput")
    gmats = nc.dram_tensor("gmats", [8, 128, 128], f32w, kind="ExternalInput")
    igc = nc.dram_tensor("igc", [4, 128, 512], f32w, kind="ExternalInput")
    wct = nc.dram_tensor("wct", [2, 128, 128], f32w, kind="ExternalInput")
    lamred = nc.dram_tensor("lamred", [2, 128, 256], f32, kind="ExternalInput")
    lamim = nc.dram_tensor("lamim", [2, 128, 128], f32, kind="ExternalInput")
    gamd = nc.dram_tensor("gamd", [128, 512], f32, kind="ExternalInput")
    biasa = nc.dram_tensor("biasa", [1, 256], f32w, kind="ExternalInput")
    biasaswr = nc.dram_tensor("biasaswr", [1, 256], f32w, kind="ExternalInput")
    e0row = nc.dram_tensor("e0row", [1, 128], f32w, kind="ExternalInput")
    lnw = nc.dram_tensor("lnw", [128, 256], f32, kind="ExternalInput")
    lnb = nc.dram_tensor("lnb", [128, 256], f32, kind="ExternalInput")
    bcre = nc.dram_tensor("bcre", [128, 1], f32, kind="ExternalInput")

    out = nc.dram_tensor("out", [L, 128, 256], f32, kind="ExternalOutput")
    hid = nc.dram_tensor("hid", [2, 128, 256], f32, kind="ExternalOutput")
    dbg = (
        nc.dram_tensor("dbg", [8, 128, 256], f32, kind="ExternalOutput")
        if _DEBUG
        else None
    )

    ID = mybir.ActivationFunctionType.Identity
    SQ = mybir.ActivationFunctionType.Square
    MUL = mybir.AluOpType.mult
    ADD = mybir.AluOpType.add

    with tile.TileContext(nc) as tc:
        with (
            tc.tile_pool(name="const", bufs=1) as cst,
            tc.tile_pool(name="xp", bufs=6) as xp,
            tc.tile_pool(name="h1p", bufs=6) as h1p,
            tc.tile_pool(name="zp", bufs=3) as zp,
            tc.tile_pool(name="yst", bufs=4) as yst,
            tc.tile_pool(name="z2p", bufs=4) as z2p,
            tc.tile_pool(name="opp", bufs=32) as opp,
            tc.tile_pool(name="scr", bufs=6) as scr,
            tc.tile_pool(name="fin", bufs=4) as fin,
            tc.tile_pool(name="ps", bufs=8, space="PSUM") as ps,
            tc.tile_pool(name="dram", bufs=2, space="DRAM") as dramp,
        ):
            # ---- load constants ----
            def cload(src, shape, tag):
                t = cst.tile(shape, f32, tag=tag, name="c_" + tag)
                nc.sync.dma_start(t[:], src)
                return t

            def wload(src, shape, tag):
                # matmul weights: DRAM already declared f32r; plain DMA
                t = cst.tile(shape, f32w, tag=tag, name="c_" + tag)
                nc.sync.dma_start(t[:], src)
                return t

            wbc_s = wload(wbc[:], [128, 256], "wbc")
            wbcsw_s = wload(wbcsw[:], [128, 256], "wbcsw")
            gre_s = {}
            ngim_s = {}
            for i in range(2):
                for o in range(2):
                    gre_s[i, o] = wload(gmats[i * 2 + o], [128, 128], f"gre{i}{o}")
                    ngim_s[i, o] = wload(gmats[4 + i * 2 + o], [128, 128], f"ngim{i}{o}")
            igc_s = [cst.tile([128, 512], f32w, tag=f"igc{k}", name=f"igcs{k}") for k in range(4)]
            for k in range(4):
                nc.sync.dma_start(igc_s[k][:], igc[k])
            wct_s = [cst.tile([128, 128], f32w, tag=f"wct{k}", name=f"wcts{k}") for k in range(2)]
            for k in range(2):
                nc.sync.dma_start(wct_s[k][:], wct[k])
            lamred_s = [cst.tile([128, 256], f32, tag=f"lamred{k}", name=f"lamreds{k}") for k in range(2)]
            for k in range(2):
                nc.sync.dma_start(lamred_s[k][:], lamred[k])
            lamim_s = [cst.tile([128, 128], f32, tag=f"lamim{k}", name=f"lamims{k}") for k in range(2)]
            for k in range(2):
                nc.sync.dma_start(lamim_s[k][:], lamim[k])
            gamd_s = cst.tile([128, 512], f32, tag="gamd", name="gamds")
            nc.sync.dma_start(gamd_s[:], gamd[:])
            biasa_s = wload(biasa[:], [1, 256], "biasa")
            biasasw_s = wload(biasaswr[:], [1, 256], "biasasw")
            e0_s = wload(e0row[:], [1, 128], "e0_s")
            lnw_s = cload(lnw[:], [128, 256], "lnw")
            lnb_s = cload(lnb[:], [128, 256], "lnb")
            bcre_s = cload(bcre[:], [128, 1], "bcre")

            ones_c = cst.tile([128, 1], f32, tag="ones_c")
            nc.vector.memset(ones_c, 1.0)
            ones_r = cst.tile([1, 128], f32, tag="ones_r")
            nc.vector.memset(ones_r, 1.0)

            stats1 = cst.tile([128, 32], f32, tag="stats1")
            stats2 = cst.tile([128, 32], f32, tag="stats2")

            # ---- scan state ----
            y0 = yst.tile([128, 256], f32, tag="y0", name="y0_init")
            y1 = yst.tile([128, 256], f32, tag="y1", name="y1_init")
            nc.vector.memset(y0, 0.0)
            nc.vector.memset(y1, 0.0)
            ystate = [y0, y1]

            op_tiles = []
            x2_tiles = []

            # ================= phase 1 =================
            # Blocks of 4 l: FFT weights load once per block (weight-outer,
            # l-inner); PSUM consolidated to one bank per (l, stage); scan
            # state double-buffered so the serial scan chain stays on DVE
            # without waiting for the PE-side IFFT reads.
            LB = 4
            rcast = (lambda ap: ap.bitcast(mybir.dt.float32r)) if use_f32r else (lambda ap: ap)
            last_y = None
            for blk in range(L // LB):
                ls = list(range(blk * LB, blk * LB + LB))
                h1s = {}
                h1sws = {}
                for l in ls:
                    xt = xp.tile([128, 256], f32w, tag="xt", name=f"xt{l}")
                    nc.sync.dma_start(xt[:], xin[l])
                    xt2 = opp.tile([128, 256], f32w, tag="x2", name=f"x2_{l}")
                    nc.sync.dma_start(xt2[:], xin[l])
                    x2_tiles.append(xt2)
                    ph = ps.tile([128, 512], f32, tag="ps", name=f"ph{l}")
                    nc.tensor.matmul(ph[:, 0:256], xt[:, 0:128], wbc_s[:],
                                     start=True, stop=False)
                    nc.tensor.matmul(ph[:, 256:512], xt[:, 128:256], wbc_s[:],
                                     start=False, stop=False)
                    nc.tensor.matmul(ph[:, 0:256], e0_s[:], biasa_s[:],
                                     start=False, stop=True)
                    phsw = ps.tile([128, 512], f32, tag="ps", name=f"phsw{l}")
                    nc.tensor.matmul(phsw[:, 0:256], xt[:, 0:128], wbcsw_s[:],
                                     start=True, stop=False)
                    nc.tensor.matmul(phsw[:, 256:512], xt[:, 128:256], wbcsw_s[:],
                                     start=False, stop=False)
                    nc.tensor.matmul(phsw[:, 0:256], e0_s[:], biasasw_s[:],
                                     start=False, stop=True)
                    h1 = h1p.tile([128, 512], f32, tag="h1", name=f"h1_{l}")
                    nc.scalar.copy(rcast(h1[:]), ph[:])
                    h1sw = h1p.tile([128, 512], f32, tag="h1sw", name=f"h1sw_{l}")
                    nc.scalar.copy(rcast(h1sw[:]), phsw[:])
                    h1s[l] = h1
                    h1sws[l] = h1sw

                zpss = {}
                for l in ls:
                    zpss[l] = ps.tile([128, 512], f32, tag="ps", name=f"zb{l}")
                # all FFT matmuls are N=256: Gre(i,o) @ h1[i] feeds [zre|zim];
                # NGim(i,o) @ h1sw[i] (= [im, -re]) adds the -Gim@im | +Gim@re terms
                for o in range(2):
                    for i in range(2):
                        for l in ls:
                            nc.tensor.matmul(
                                zpss[l][:, o * 256 : (o + 1) * 256],
                                gre_s[i, o][:],
                                rcast(h1s[l][:, i * 256 : (i + 1) * 256]),
                                start=(o == 0 and i == 0), stop=False,
                            )
                for o in range(2):
                    for i in range(2):
                        for l in ls:
                            nc.tensor.matmul(
                                zpss[l][:, o * 256 : (o + 1) * 256],
                                ngim_s[i, o][:],
                                rcast(h1sws[l][:, i * 256 : (i + 1) * 256]),
                                start=False, stop=(i == 1),
                            )

                z2s = {}
                for l in ls:
                    zt = zp.tile([128, 512], f32, tag="z", name=f"zt{l}")
                    nc.vector.tensor_mul(zt[:], zpss[l][:], gamd_s[:])

                    newy = [
                        yst.tile([128, 256], f32, tag=f"y{k}", name=f"y{k}_{l}")
                        for k in range(2)
                    ]
                    for k in range(2):
                        yo = ystate[k]
                        t1 = scr.tile([128, 256], f32, tag=f"t1_{k}", name=f"t1_{k}_{l}")
                        t2r = scr.tile([128, 128], f32, tag=f"t2r_{k}", name=f"t2r_{k}_{l}")
                        t2i = scr.tile([128, 128], f32, tag=f"t2i_{k}", name=f"t2i_{k}_{l}")
                        nc.vector.tensor_mul(t1[:], yo[:], lamred_s[k][:])
                        nc.gpsimd.tensor_mul(t2r[:], yo[:, 128:256], lamim_s[k][:])
                        nc.gpsimd.tensor_mul(t2i[:], yo[:, 0:128], lamim_s[k][:])
                        nc.vector.tensor_sub(rcast(newy[k][:, 0:128]), t1[:, 0:128], t2r[:])
                        nc.vector.tensor_add(rcast(newy[k][:, 128:256]), t1[:, 128:256], t2i[:])
                        nc.vector.tensor_add(
                            rcast(newy[k][:]), newy[k][:], zt[:, k * 256 : (k + 1) * 256]
                        )
                    ystate = newy

                    if _DEBUG and l == 0:
                        nc.sync.dma_start(dbg[0], h1s[l][:, 0:256])
                        nc.sync.dma_start(dbg[1], h1s[l][:, 256:512])
                        nc.sync.dma_start(dbg[2], zt[:, 0:256])
                        nc.sync.dma_start(dbg[3], zt[:, 256:512])
                        nc.sync.dma_start(dbg[4], ystate[0][:])
                        nc.sync.dma_start(dbg[5], ystate[1][:])
                    if _DEBUG and l == 1:
                        nc.sync.dma_start(dbg[6], ystate[0][:])
                        nc.sync.dma_start(dbg[7], ystate[1][:])
                    if l == L - 1:
                        nc.sync.dma_start(hid[0], ystate[0][:])
                        nc.sync.dma_start(hid[1], ystate[1][:])

                    z2ps = ps.tile([128, 512], f32, tag="ps", name=f"z2_{l}")
                    nc.tensor.matmul(z2ps[:], rcast(ystate[0][:, 0:128]), igc_s[0][:],
                                     start=True, stop=False)
                    nc.tensor.matmul(z2ps[:], rcast(ystate[1][:, 0:128]), igc_s[1][:],
                                     start=False, stop=False)
                    nc.tensor.matmul(z2ps[:], rcast(ystate[0][:, 128:256]), igc_s[2][:],
                                     start=False, stop=False)
                    nc.tensor.matmul(z2ps[:], rcast(ystate[1][:, 128:256]), igc_s[3][:],
                                     start=False, stop=True)
                    z2 = z2p.tile([128, 512], f32, tag="z2", name=f"z2sb_{l}")
                    nc.scalar.copy(rcast(z2[:]), z2ps[:])
                    z2s[l] = z2

                op_ps = [
                    ps.tile([128, 512], f32, tag="ps", name=f"opb{blk}_{j}")
                    for j in range(LB // 2)
                ]
                for wi in range(2):
                    for jj, l in enumerate(ls):
                        bank = op_ps[jj // 2]
                        off = (jj % 2) * 256
                        nc.tensor.matmul(
                            bank[:, off : off + 256],
                            wct_s[wi][:],
                            rcast(z2s[l][:, wi * 256 : (wi + 1) * 256]),
                            start=(wi == 0 and jj % 2 == 0),
                            stop=(wi == 1),
                        )
                for jj, l in enumerate(ls):
                    bank = op_ps[jj // 2]
                    off = (jj % 2) * 256
                    op_sb = opp.tile([128, 256], f32, tag="op", name=f"op_{l}")
                    nc.scalar.activation(
                        op_sb[:], bank[:, off : off + 256], ID,
                        bias=bcre_s[:, 0:1], scale=1.0,
                        accum_out=stats1[:, l : l + 1],
                    )
                    sq = scr.tile([128, 256], f32, tag="sq", name=f"sq_{l}")
                    nc.gpsimd.tensor_mul(sq[:], op_sb[:], op_sb[:])
                    nc.vector.reduce_sum(
                        stats2[:, l : l + 1], sq[:],
                        axis=mybir.AxisListType.X,
                    )
                    op_tiles.append(op_sb)

            # ================= phase 2: LN stats =================
            st_ps = ps.tile([1, 64], f32, tag="ps")
            nc.tensor.matmul(st_ps[:, 0:32], ones_c[:], stats1[:], start=True, stop=False)
            nc.tensor.matmul(st_ps[:, 32:64], ones_c[:], stats2[:], start=False, stop=True)
            srow = cst.tile([1, 64], f32, tag="srow")
            nc.scalar.copy(srow[:], st_ps[:])

            cin = dramp.tile([1, 64], f32, tag="cin")
            cout = dramp.tile([1, 64], f32, tag="cout")
            nc.sync.dma_start(cin[:], srow[:])
            if _SIM:
                nc.sync.dma_start(cout[:], cin[:])
            else:
                nc.gpsimd.collective_compute(
                    "AllReduce",
                    ADD,
                    replica_groups=[[0, 1, 2, 3], [4, 5, 6, 7]],
                    ins=[cin.opt()],
                    outs=[cout.opt()],
                )
            grow = cst.tile([1, 64], f32, tag="grow")
            nc.sync.dma_start(grow[:], cout[:])

            mrow = cst.tile([1, 32], f32, tag="mrow")
            vrow = cst.tile([1, 32], f32, tag="vrow")
            m2row = cst.tile([1, 32], f32, tag="m2row")
            arow = cst.tile([1, 32], f32, tag="arow")
            brow = cst.tile([1, 32], f32, tag="brow")
            nc.scalar.mul(mrow[:], grow[:, 0:32], 1.0 / LN_N)
            nc.scalar.mul(vrow[:], grow[:, 32:64], 1.0 / LN_N)
            nc.vector.tensor_mul(m2row[:], mrow[:], mrow[:])
            nc.vector.tensor_sub(vrow[:], vrow[:], m2row[:])
            eps_t = cst.tile([1, 1], f32, tag="eps_t")
            nc.vector.memset(eps_t, 1e-5)
            nc.scalar.activation(
                vrow[:], vrow[:], mybir.ActivationFunctionType.Sqrt,
                bias=eps_t[:, 0:1], scale=1.0,
            )
            nc.vector.reciprocal(arow[:], vrow[:])
            nc.vector.tensor_mul(brow[:], mrow[:], arow[:])
            nc.scalar.mul(brow[:], brow[:], -1.0)

            abrow = cst.tile([1, 64], f32, tag="abrow")
            nc.scalar.copy(abrow[:, 0:32], arow[:])
            nc.scalar.copy(abrow[:, 32:64], brow[:])
            ab_ps = ps.tile([128, 64], f32, tag="ps")
            nc.tensor.matmul(ab_ps[:], ones_r[:], abrow[:], start=True, stop=True)
            ab_sb = cst.tile([128, 64], f32, tag="ab_sb")
            nc.scalar.copy(ab_sb[:], ab_ps[:])

            # ========= phase 3: normalize + residual (DVE/GpSimd split) =========
            for l in range(L):
                eng = nc.vector if l % 2 == 0 else nc.gpsimd
                xt2 = x2_tiles[l]
                t = fin.tile([128, 256], f32, tag="t", name=f"t_{l}")
                eng.tensor_scalar(
                    out=t[:], in0=op_tiles[l][:],
                    scalar1=ab_sb[:, l : l + 1],
                    scalar2=ab_sb[:, 32 + l : 33 + l],
                    op0=MUL, op1=ADD,
                )
                if not ln_trivial:
                    eng.tensor_mul(t[:], t[:], lnw_s[:])
                    eng.tensor_add(t[:], t[:], lnb_s[:])
                eng.tensor_add(t[:], t[:], xt2[:].bitcast(f32) if use_f32r else xt2[:])
                nc.sync.dma_start(out[l], t[:])

    _cap_sync_waits(nc)
    return nc


_NC = {}
_USE_F32R = not bool(os.environ.get("K_F32"))


def _get_nc(ln_trivial=False):
    _patch_tail_drain()
    key = (ln_trivial, _USE_F32R)
    if key not in _NC:
        _NC[key] = _build_nc(ln_trivial, use_f32r=_USE_F32R)
    return _NC[key]


def _make_inputs(inputs):
    """Host-side constant precompute + per-core sharding."""
    x = np.asarray(inputs["x"], np.float32)
    params_log = np.asarray(inputs["params_log"], np.float64)
    nu = np.exp(params_log[:C])
    theta = np.exp(params_log[C : 2 * C])
    gamma = np.exp(params_log[2 * C :])
    lam = np.exp(-nu + 1j * theta)

    Wb_re = np.asarray(inputs["Wb_re"], np.float32)
    Wb_im = np.asarray(inputs["Wb_im"], np.float32)
    Wc_re = np.asarray(inputs["Wc_re"], np.float32)
    Wc_im = np.asarray(inputs["Wc_im"], np.float32)

    idx = np.arange(H)
    F = np.exp(-2j * np.pi * np.outer(idx, idx) / H)
    IF = np.exp(+2j * np.pi * np.outer(idx, idx) / H) / H

    I8 = np.eye(8, dtype=np.float64)

    def kron8(A):
        return np.kron(A, I8).astype(np.float32)

    gm = np.zeros((8, 128, 128), np.float32)
    for i in range(2):
        for o in range(2):
            Fs = F[o * 16 : (o + 1) * 16, i * 16 : (i + 1) * 16]
            gm[i * 2 + o] = kron8(Fs.real.T)
            gm[4 + i * 2 + o] = -kron8(Fs.imag.T)

    igr = []
    igi = []
    for half in range(2):
        Bm = IF[:, half * 16 : (half + 1) * 16].T  # [16 hf, 32 h]
        igr.append(kron8(Bm.real))
        igi.append(kron8(Bm.imag))
    igc = np.zeros((4, 128, 512), np.float32)
    igc[0] = np.concatenate([igr[0], igi[0]], 1)
    igc[1] = np.concatenate([igr[1], igi[1]], 1)
    igc[2] = np.concatenate([-igi[0], igr[0]], 1)
    igc[3] = np.concatenate([-igi[1], igr[1]], 1)

    wbc = np.concatenate([Wb_re.T, Wb_im.T], 1).astype(np.float32)
    wbcsw = np.concatenate([Wb_im.T, -Wb_re.T], 1).astype(np.float32)
    wct = np.stack([Wc_re.T, -Wc_im.T]).astype(np.float32)

    def plane(v, half):  # [C,H] -> [128,128]: P[hl*8+w, c] = v[c, half*16+hl]
        return np.repeat(
            v[:, half * 16 : (half + 1) * 16].T, 8, axis=0
        ).astype(np.float32)

    lamred = np.stack(
        [np.concatenate([plane(lam.real, k)] * 2, 1) for k in range(2)]
    )
    lamim = np.stack([plane(lam.imag, k) for k in range(2)])
    gamd = np.concatenate(
        [np.concatenate([plane(gamma, k)] * 2, 1) for k in range(2)], 1
    )

    bcre = np.asarray(inputs["bc_re"], np.float32).reshape(128, 1)

    ln_w = np.asarray(inputs["ln_w"], np.float32)
    ln_b = np.asarray(inputs["ln_b"], np.float32)

    in_maps = []
    for core in range(N_CORES):
        b, wg = divmod(core, 4)
        sl = slice(wg * 8, (wg + 1) * 8)
        biasa = np.zeros((1, 256), np.float32)
        biasasw = np.zeros((1, 256), np.float32)
        if wg == 0:
            biasa[0, 0:128] = inputs["bb_re"]
            biasa[0, 128:256] = inputs["bb_im"]
            biasasw[0, 0:128] = inputs["bb_im"]
            biasasw[0, 128:256] = -np.asarray(inputs["bb_re"])
        in_maps.append(
            dict(
                x=np.ascontiguousarray(
                    x[b, :, :, :, sl].reshape(L, 128, 256)
                ),
                wbc=wbc,
                wbcsw=wbcsw,
                biasaswr=biasasw,
                e0row=np.eye(1, 128, dtype=np.float32),
                gmats=gm,
                igc=igc,
                wct=wct,
                lamred=lamred.astype(np.float32),
                lamim=lamim.astype(np.float32),
                gamd=gamd.astype(np.float32),
                biasa=biasa,
                lnw=np.ascontiguousarray(ln_w[:, :, sl].reshape(128, 256)),
                lnb=np.ascontiguousarray(ln_b[:, :, sl].reshape(128, 256)),
                bcre=bcre,
            )
        )
    return in_maps


def _run(inputs, trace=False, trace_kwargs=None):
    from concourse.bass_utils import run_bass_kernel_spmd

    ln_trivial = bool(
        np.all(np.asarray(inputs["ln_w"]) == 1.0)
        and np.all(np.asarray(inputs["ln_b"]) == 0.0)
    )
    nc = _get_nc(ln_trivial)
    in_maps = _make_inputs(inputs)
    kw = {}
    if trace:
        kw["trace"] = True
        if trace_kwargs:
            kw["trace_kwargs"] = trace_kwargs
    res = run_bass_kernel_spmd(nc, in_maps, core_ids=list(range(N_CORES)), **kw)

    x = np.asarray(inputs["x"], np.float32)
    out_full = np.zeros((B, L, C, H, W), np.float32)
    yfreq = np.zeros((B, C, H, W), np.complex64)
    for core in range(N_CORES):
        b, wg = divmod(core, 4)
        sl = slice(wg * 8, (wg + 1) * 8)
        r = res.results[core]
        out_full[b, :, :, :, sl] = r["out"].reshape(L, C, H, 8)
        hv = r["hid"]  # [2,128,256] = [half, (hl,w), (re c|im c)]
        cv = hv[:, :, 0:128] + 1j * hv[:, :, 128:256]  # [2,128,128]
        yfreq[b, :, :, sl] = (
            cv.reshape(2, 16, 8, 128).transpose(3, 0, 1, 2).reshape(C, H, 8)
        )
    last_hidden = np.fft.fft(yfreq, axis=-1)[:, None].astype(np.complex64)
    return (out_full, last_hidden), res


def kernel(**inputs):
    (out_full, last_hidden), _ = _run(inputs)
    return out_full, last_hidden
